# revision 1
# baseline (speedup 1.0000x reference)
"""ChebyKANLinear Trainium2 kernel.

Math: y[b,o] = (1/I) * sum_{i,d} T_d(c[b,i]) * W[i,o,d],  c = tanh(x)
with Chebyshev T_0=1, T_1=c, T_2=2c^2-1, T_3=4c^3-3c.
(The reference also clips c to [-1+1e-7, 1-1e-7] before arccos; in the
monomial form below the bound is numerically irrelevant — |tanh|max for this
input distribution is 0.99992, far below it — so the clip is dropped.)

Re-expressed in the monomial basis (exact linear recombination, folded into
the weights on the host):
    y = bias + c @ V1 + c^2 @ V2 + c^3 @ V3
    V1 = (W1 - 3*W3)/I, V2 = 2*W2/I, V3 = 4*W3/I, bias_o = sum_i (W0 - W2)[i,o]/I

Sharding: 2D — batch into 4 shards x output_dim into 2 shards across the 8
NeuronCores. Per core the matmuls are computed TRANSPOSED,
    yT[o, b] = sum_k  V_k[i, o].T @ (c^k)[i, b]
so each core runs only 6 fp32 matmuls of [K=128, M=128, N=512] (N=512 is the
fp32 moving-operand max — fewest PE passes for this contraction), and the
bias becomes a per-partition scalar fused into the PSUM->SBUF copy
(vector.tensor_scalar_add) instead of costing extra matmuls.

Perf notes baked in from trace analysis:
- All of V plus the bias column ride ONE wide-row dma_start ([128, 769] ->
  3KB/partition rows); narrow-row DMAs measured ~3x slower per byte.
- x rides two dma_starts on the other HWDGE queue (sync/SP).
- Two real-shaped (K=128, N=512) warmup matmuls on memset tiles run during
  the DMA phase so the PE HAM clock-gate (1.2 -> 2.4 GHz) opens right as the
  real accumulation chain peaks.
- Output is written as two half DMAs on the two queues to overlap the
  PSUM->SBUF bias-add with the store.
"""

from contextlib import ExitStack

import numpy as np

import concourse.bass as bass
import concourse.tile as tile
from concourse import bacc, mybir
from concourse.bass_utils import run_bass_kernel_spmd

N_CORES = 8
B, I, O, D = 2048, 256, 256, 4
RB, SO = 4, 2  # batch shards x output shards
BL = B // RB  # 512 batch rows per core
OL = O // SO  # 128 output cols per core
F32 = mybir.dt.float32

_cache = {}


def _build_program():
    nc = bacc.Bacc("TRN2", target_bir_lowering=False, debug=False, num_devices=N_CORES)

    # [i_half, i_in_half, b_local]  (x slice pre-transposed on host)
    xt_d = nc.dram_tensor("xt", [2, 128, BL], F32, kind="ExternalInput")
    # packed weights: col (ih*3+d)*OL + o holds V[d, ih*128+i, o]; col 768 = bias
    vb_d = nc.dram_tensor("vb", [128, 6 * OL + 1], F32, kind="ExternalInput")
    # transposed output [o_local, b_local]
    y_d = nc.dram_tensor("y", [OL, BL], F32, kind="ExternalOutput")

    with tile.TileContext(nc) as tc, ExitStack() as ctx:
        pool = ctx.enter_context(tc.tile_pool(name="main", bufs=1))
        psum = ctx.enter_context(
            tc.tile_pool(name="psum", bufs=1, space=bass.MemorySpace.PSUM)
        )

        # PE warmup operands (DVE is idle this early; values are irrelevant)
        wu_w = pool.tile([128, 128], F32, tag="wu_w")
        nc.vector.memset(wu_w[:], 1.0)
        wu_r = pool.tile([128, 512], F32, tag="wu_r")
        nc.vector.memset(wu_r[:], 1.0)

        # One dma_start per tensor; x pair on the sync queue (the scalar
        # HWDGE queue measured ~1us slower to first byte), packed V+bias on
        # scalar. Splitting tensors across queues and other rebalances all
        # measured slower (per-dma fixed cost + queue startup).
        vb = pool.tile([128, 6 * OL + 1], F32, tag="vb")
        nc.scalar.dma_start(vb[:], vb_d[:])
        xt = {}
        for ih in range(2):
            xt[ih] = pool.tile([128, BL], F32, tag=f"xt{ih}", name=f"xt{ih}")
        hb = BL // 2
        nc.sync.dma_start(xt[0][:, :hb], xt_d[0, :, :hb])
        nc.sync.dma_start(xt[0][:, hb:], xt_d[0, :, hb:])
        nc.sync.dma_start(xt[1][:], xt_d[1])

        # Two warmup matmuls: dense K=128 N=512 so the HAM clock-gate sees
        # real PE activity; they end right as the real chain starts (a gap
        # would reset the HAM busy-window progress — measured).
        wu_acc = psum.tile([128, 512], F32, tag="wu_acc")
        nc.tensor.matmul(wu_acc[:], wu_w[:], wu_r[:], start=True, stop=True)
        # second warmup at half width: keeps the PE streaming right up to
        # the (earlier-starting) real chain without delaying it
        nc.tensor.matmul(
            wu_acc[:, :256], wu_w[:], wu_r[:, :256], start=True, stop=True
        )

        # basis: c = tanh(xT) on ACT, c^2/c^3 on DVE
        basis = {}
        c0 = pool.tile([128, BL], F32, tag="c0")
        nc.scalar.activation(
            c0[:, :hb], xt[0][:, :hb], mybir.ActivationFunctionType.Tanh
        )
        nc.scalar.activation(
            c0[:, hb:], xt[0][:, hb:], mybir.ActivationFunctionType.Tanh
        )
        basis[(0, 0)] = c0
        c1 = pool.tile([128, BL], F32, tag="c1")
        nc.scalar.activation(c1[:], xt[1][:], mybir.ActivationFunctionType.Tanh)
        basis[(0, 1)] = c1
        for ih in range(2):
            c2 = pool.tile([128, BL], F32, tag=f"c2{ih}")
            nc.vector.tensor_mul(c2[:], basis[(0, ih)][:], basis[(0, ih)][:])
            basis[(1, ih)] = c2
        for ih in range(2):
            c3 = pool.tile([128, BL], F32, tag=f"c3{ih}")
            nc.vector.tensor_mul(c3[:], basis[(1, ih)][:], basis[(0, ih)][:])
            basis[(2, ih)] = c3

        # yT[o, b] accumulation: 6 matmuls alternating between TWO PSUM
        # banks (ih=0 -> acc_a, ih=1 -> acc_b) so consecutive accumulating
        # passes don't serialize on one bank; merged + bias in one fused
        # DVE op per half: (acc_a + bias) + acc_b.
        acc_a = psum.tile([128, BL], F32, tag="acc_a")
        acc_b = psum.tile([128, BL], F32, tag="acc_b")
        accs = {0: acc_a, 1: acc_b}
        nc.tensor.matmul(
            acc_a[:OL, :hb], vb[:, :OL], c0[:, :hb], start=True, stop=False
        )
        # start=False: half0's start already cleared the whole bank's
        # has_written bits; a second start would wipe half0's state
        nc.tensor.matmul(
            acc_a[:OL, hb:], vb[:, :OL], c0[:, hb:], start=False, stop=False
        )
        # ordered by operand readiness: c2_0 lands before tanh1 does, so
        # the in-order PE must see (1,0) ahead of (0,1) or it stalls
        mm_order = [(1, 0), (0, 1), (2, 0), (1, 1), (2, 1)]
        for d, ih in mm_order:
            col = (ih * 3 + d) * OL
            nc.tensor.matmul(
                accs[ih][:OL, :],
                vb[:, col : col + OL],
                basis[(d, ih)][:],
                start=(d == 0),
                stop=(d == 2),
            )

        # DVE can read only ONE PSUM operand per op: pre-merge acc_a + bias
        # into SBUF (overlaps the final acc_b matmul), then y = tmp + acc_b.
        bias_col = vb[:, 6 * OL : 6 * OL + 1]
        tmp_sb = pool.tile([OL, BL], F32, tag="tmp_sb")
        y_sb = pool.tile([OL, BL], F32, tag="y_sb")
        half = BL // 2
        nc.vector.tensor_scalar_add(tmp_sb[:, :half], acc_a[:OL, :half], bias_col)
        nc.vector.tensor_scalar_add(tmp_sb[:, half:], acc_a[:OL, half:], bias_col)
        q = BL // 4
        for k in range(4):
            s = slice(k * q, (k + 1) * q)
            nc.vector.tensor_tensor(
                y_sb[:, s], acc_b[:OL, s], tmp_sb[:, s], mybir.AluOpType.add
            )
            (nc.sync if k % 2 == 0 else nc.scalar).dma_start(y_d[:, s], y_sb[:, s])

    nc.compile()
    return nc


def _get_program():
    if "nc" not in _cache:
        _cache["nc"] = _build_program()
    return _cache["nc"]


def _make_in_maps(x, cheby_coeffs):
    x = np.ascontiguousarray(x, dtype=np.float32)
    W = np.ascontiguousarray(cheby_coeffs, dtype=np.float32)
    assert x.shape == (B, I) and W.shape == (I, O, D)

    inv_i = np.float32(1.0 / I)
    V = np.stack(
        [
            W[:, :, 1] - 3.0 * W[:, :, 3],
            2.0 * W[:, :, 2],
            4.0 * W[:, :, 3],
        ]
    ).astype(np.float32) * inv_i  # [3, I, O]
    bias_full = (W[:, :, 0] - W[:, :, 2]).sum(axis=0, dtype=np.float32) * inv_i  # [O]

    xt_shards = []
    for rb in range(RB):
        xs = x[rb * BL : (rb + 1) * BL, :]  # [BL, I]
        xt_shards.append(np.ascontiguousarray(xs.T).reshape(2, 128, BL))
    vb_shards = []
    for so in range(SO):
        vb = np.empty((128, 6 * OL + 1), dtype=np.float32)
        for ih in range(2):
            for d in range(3):
                col = (ih * 3 + d) * OL
                # vb[i, col+o] = V[d, ih*128+i, so*OL+o]
                vb[:, col : col + OL] = V[
                    d, ih * 128 : (ih + 1) * 128, so * OL : (so + 1) * OL
                ]
        vb[:, 6 * OL] = bias_full[so * OL : (so + 1) * OL]
        vb_shards.append(vb)
    in_maps = []
    for c_id in range(N_CORES):
        rb, so = divmod(c_id, SO)
        in_maps.append({"xt": xt_shards[rb], "vb": vb_shards[so]})
    return in_maps


def kernel(x, cheby_coeffs):
    nc = _get_program()
    in_maps = _make_in_maps(x, cheby_coeffs)
    res = run_bass_kernel_spmd(nc, in_maps, list(range(N_CORES)))
    y = np.empty((B, O), dtype=np.float32)
    for c_id in range(N_CORES):
        rb, so = divmod(c_id, SO)
        y[rb * BL : (rb + 1) * BL, so * OL : (so + 1) * OL] = res.results[c_id]["y"].T
    return y



# revision 4
# speedup vs baseline: 1.2137x; 1.2137x over previous
"""ChebyKANLinear Trainium2 kernel (fp16 pipeline).

Math: y[b,o] = (1/I) * sum_{i,d} T_d(c[b,i]) * W[i,o,d],  c = tanh(x)
with Chebyshev T_0=1, T_1=c, T_2=2c^2-1, T_3=4c^3-3c.
Monomial re-expression (exact linear recombination, folded on the host):
    y = (bias_u + c @ V1 + c^2 @ V2 + c^3 @ V3) / I
    V1 = W1 - 3*W3, V2 = 2*W2, V3 = 4*W3, bias_u[o] = sum_i (W0 - W2)[i,o]
V is deliberately NOT pre-divided by I: the unscaled values (std ~3e-3) sit
comfortably in fp16 normal range (V/I ~1e-5 would be subnormal), and the
1/I rides the final fused PSUM->SBUF op: y = (acc + bias_u) * (1/I).

Everything 16-bit where the 2e-2 rel-err budget allows (measured host-sim
rel err 7.6e-4): x, c, c^2, c^3, V, bias and the y output travel as fp16
(host converts y back to fp32); only PSUM accumulation is fp32. This
halves HBM<->SBUF traffic vs fp32 and runs each matmul as ONE PE pass
(fp32 needed a LOW+HIGH pair), cutting the accumulation chain ~3x.

Sharding: 2D - batch into 4 shards x output_dim into 2 shards across the 8
NeuronCores. Per core the matmuls are computed TRANSPOSED,
    yT[o, b] = sum_k  V_k[i, o].T @ (c^k)[i, b]
so each core runs 6 fp16 matmuls of [K=128, M=128, N=512].

Perf notes baked in from trace analysis:
- xt rides ONE wide-row dma_start ([128, 1024] fp16 -> 2KB rows) on the
  sync queue; vb+bias ride one [128, 769] fp16 dma on the scalar queue.
  Narrow-row DMAs measured ~3x slower per byte; extra descriptors cost
  ~620ns issue each.
- Two real-shaped warmup matmuls on memset tiles run during the DMA phase
  so the PE HAM clock-gate (1.2 -> 2.4 GHz) opens right as the real
  accumulation chain starts.
- All 6 matmuls accumulate into ONE PSUM bank (trace showed same-bank
  back-to-back accumulation runs at full pass rate), so the epilogue is a
  single fused op per half: y = (acc + bias_u) * (1/I) -> fp16 SBUF,
  stored as two half DMAs on the two HWDGE queues.
"""

from contextlib import ExitStack

import numpy as np

import concourse.bass as bass
import concourse.tile as tile
from concourse import bacc, mybir
from concourse.bass_utils import run_bass_kernel_spmd

N_CORES = 8
B, I, O, D = 2048, 256, 256, 4
RB, SO = 4, 2  # batch shards x output shards
BL = B // RB  # 512 batch rows per core
OL = O // SO  # 128 output cols per core
F16 = mybir.dt.float16
F32 = mybir.dt.float32
INV_I = 1.0 / I

_cache = {}


def _build_program():
    nc = bacc.Bacc("TRN2", target_bir_lowering=False, debug=False, num_devices=N_CORES)

    # [i_half(0..127), ih*BL + b_local]  (x slice pre-transposed on host)
    xt_d = nc.dram_tensor("xt", [128, 2 * BL], F16, kind="ExternalInput")
    # packed weights: col (ih*3+d)*OL + o holds V[d, ih*128+i, o]; col 768 =
    # unscaled bias (broadcast along partitions)
    vb_d = nc.dram_tensor("vb", [128, 6 * OL + 1], F16, kind="ExternalInput")
    # transposed fp16 output [o_local, b_local]
    y_d = nc.dram_tensor("y", [OL, BL], F16, kind="ExternalOutput")

    with tile.TileContext(nc) as tc, ExitStack() as ctx:
        pool = ctx.enter_context(tc.tile_pool(name="main", bufs=1))
        psum = ctx.enter_context(
            tc.tile_pool(name="psum", bufs=1, space=bass.MemorySpace.PSUM)
        )

        # PE warmup operands (DVE is idle this early; values are irrelevant)
        wu_w = pool.tile([128, 128], F16, tag="wu_w")
        nc.vector.memset(wu_w[:], 1.0)
        wu_r = pool.tile([128, 512], F16, tag="wu_r")
        nc.vector.memset(wu_r[:], 1.0)

        # One dma_start per tensor on separate HWDGE queues.
        vb = pool.tile([128, 6 * OL + 1], F16, tag="vb")
        nc.scalar.dma_start(vb[:], vb_d[:])
        xt = pool.tile([128, 2 * BL], F16, tag="xt")
        nc.sync.dma_start(xt[:], xt_d[:])

        # Warmup matmuls: dense K=128 N=512 so the HAM clock-gate sees real
        # PE activity; they end right as the real chain starts.
        wu_acc = psum.tile([128, 512], F32, tag="wu_acc")
        nc.tensor.matmul(wu_acc[:], wu_w[:], wu_r[:], start=True, stop=True)
        nc.tensor.matmul(
            wu_acc[:, :256], wu_w[:], wu_r[:, :256], start=True, stop=True
        )

        # basis: c = tanh(xT) on ACT (chunked for pipelining), c^2/c^3 on DVE
        hb = BL // 2
        basis = {}
        c0 = pool.tile([128, BL], F16, tag="c0")
        nc.scalar.activation(
            c0[:, :hb], xt[:, :hb], mybir.ActivationFunctionType.Tanh
        )
        nc.scalar.activation(
            c0[:, hb:], xt[:, hb:BL], mybir.ActivationFunctionType.Tanh
        )
        basis[(0, 0)] = c0
        c1 = pool.tile([128, BL], F16, tag="c1")
        nc.scalar.activation(c1[:], xt[:, BL:], mybir.ActivationFunctionType.Tanh)
        basis[(0, 1)] = c1
        for ih in range(2):
            c2 = pool.tile([128, BL], F16, tag=f"c2{ih}")
            nc.vector.tensor_mul(c2[:], basis[(0, ih)][:], basis[(0, ih)][:])
            basis[(1, ih)] = c2
        for ih in range(2):
            c3 = pool.tile([128, BL], F16, tag=f"c3{ih}")
            nc.vector.tensor_mul(c3[:], basis[(1, ih)][:], basis[(0, ih)][:])
            basis[(2, ih)] = c3

        # yT[o, b] accumulation: 6 single-pass fp16 matmuls into ONE PSUM
        # bank, ordered by operand readiness.
        acc = psum.tile([128, BL], F32, tag="acc")
        nc.tensor.matmul(
            acc[:OL, :hb], vb[:, :OL], c0[:, :hb], start=True, stop=False
        )
        # start=False: half0's start already cleared the whole bank's
        # has_written bits; a second start would wipe half0's state
        nc.tensor.matmul(
            acc[:OL, hb:], vb[:, :OL], c0[:, hb:], start=False, stop=False
        )
        mm_order = [(1, 0), (0, 1), (2, 0), (1, 1), (2, 1)]
        for d, ih in mm_order:
            col = (ih * 3 + d) * OL
            nc.tensor.matmul(
                acc[:OL, :],
                vb[:, col : col + OL],
                basis[(d, ih)][:],
                start=False,
                stop=(d == 2 and ih == 1),
            )

        # tensor_scalar's ptr operand must be fp32: upconvert the fp16 bias
        # column on the (otherwise idle) GpSimd engine right after vb lands.
        bias_col = pool.tile([128, 1], F32, tag="bias32")
        nc.gpsimd.tensor_scalar(
            bias_col[:],
            vb[:, 6 * OL : 6 * OL + 1],
            0.0,
            None,
            mybir.AluOpType.add,
        )

        # Fused epilogue per half: y = (acc + bias_u) * (1/I) -> fp16 SBUF,
        # then one half-store per HWDGE queue.
        y_sb = pool.tile([OL, BL], F16, tag="y_sb")
        for k, eng in ((0, nc.sync), (1, nc.scalar)):
            s = slice(k * hb, (k + 1) * hb)
            nc.vector.tensor_scalar(
                y_sb[:, s],
                acc[:OL, s],
                bias_col[:],
                INV_I,
                mybir.AluOpType.add,
                mybir.AluOpType.mult,
            )
            eng.dma_start(y_d[:, s], y_sb[:, s])

    nc.compile()
    return nc


def _get_program():
    if "nc" not in _cache:
        _cache["nc"] = _build_program()
    return _cache["nc"]


def _make_in_maps(x, cheby_coeffs):
    x = np.ascontiguousarray(x, dtype=np.float32)
    W = np.ascontiguousarray(cheby_coeffs, dtype=np.float32)
    assert x.shape == (B, I) and W.shape == (I, O, D)

    V = np.stack(
        [
            W[:, :, 1] - 3.0 * W[:, :, 3],
            2.0 * W[:, :, 2],
            4.0 * W[:, :, 3],
        ]
    ).astype(np.float16)  # [3, I, O] unscaled
    bias_u = (W[:, :, 0] - W[:, :, 2]).sum(axis=0, dtype=np.float32)  # [O] unscaled

    xt_shards = []
    for rb in range(RB):
        xs = x[rb * BL : (rb + 1) * BL, :].T.astype(np.float16)  # [I, BL]
        # [128, 2*BL]: cols [ih*BL + b] hold x[b, ih*128 + i]
        xt_shards.append(
            np.ascontiguousarray(np.concatenate([xs[:128], xs[128:]], axis=1))
        )
    vb_shards = []
    for so in range(SO):
        vb = np.empty((128, 6 * OL + 1), dtype=np.float16)
        for ih in range(2):
            for d in range(3):
                col = (ih * 3 + d) * OL
                vb[:, col : col + OL] = V[
                    d, ih * 128 : (ih + 1) * 128, so * OL : (so + 1) * OL
                ]
        vb[:, 6 * OL] = bias_u[so * OL : (so + 1) * OL].astype(np.float16)
        vb_shards.append(vb)
    in_maps = []
    for c_id in range(N_CORES):
        rb, so = divmod(c_id, SO)
        in_maps.append({"xt": xt_shards[rb], "vb": vb_shards[so]})
    return in_maps


def kernel(x, cheby_coeffs):
    nc = _get_program()
    in_maps = _make_in_maps(x, cheby_coeffs)
    res = run_bass_kernel_spmd(nc, in_maps, list(range(N_CORES)))
    y = np.empty((B, O), dtype=np.float32)
    for c_id in range(N_CORES):
        rb, so = divmod(c_id, SO)
        y[rb * BL : (rb + 1) * BL, so * OL : (so + 1) * OL] = (
            res.results[c_id]["y"].T.astype(np.float32)
        )
    return y
